# revision 11
# baseline (speedup 1.0000x reference)
"""Trainium2 Bass kernel for nn_AttentionModule (dual spatial/temporal attention).

Math (heads collapse since scores sum over h AND d): two rounds of single-head
attention over 64-token groups with feature dim 256, scale 1/8, shared weights,
residuals. Layer 1 groups = (b,t) over n; layer 2 groups = (b,n) over t.

Sharding: data-parallel over batch, 8 batches per core, no communication.

Per-core dataflow (per batch, feature-major activations on chip):
  x (token-major, DMA) -> PE-transpose -> XT (f32r)
  QT/KT = W-stationary fp32r matmuls + bias -> bf16
  V_tok = X-stationary fp32r matmuls (token-major; bv folded into output bias)
  S = QT'KT (bf16, pair-packed 128x128), softmax via Exp+accum_out (no max
  subtraction; logits are O(10) so fp32 exp is safe), P block-diag bf16,
  PT = PE transpose, A.T = V_tok' PT (bf16 -> fp32 PSUM)
  xsT = Wo-stationary fp32r matmul + X.T residual + (bo + Wo bv) bias
  Layer 2 identical with strided (time-major) group APs; final output is
  produced token-major by accumulating A-stationary matmuls with PE-transposed
  xsT residual slices in one PSUM group, so no output transpose pass is needed.
"""
import sys

if "/opt/trn_rl_repo" not in sys.path:
    sys.path.insert(0, "/opt/trn_rl_repo")

import numpy as np
import ml_dtypes

import concourse.bass as bass
import concourse.tile as tile
import concourse.mybir as mybir

F32 = mybir.dt.float32
F32R = mybir.dt.float32r
BF16 = mybir.dt.bfloat16
AF = mybir.ActivationFunctionType

N_CORES = 8
B_FULL, T, N, F = 64, 64, 64, 256
NB = B_FULL // N_CORES          # batches per core
TOK = T * N                     # tokens per batch (4096)
NPAIR = TOK // 128              # 32 pairs of 64-token groups per batch


def _split_waits(nc, maxw=1):
    """This walrus build accepts at most one sync-wait per instruction; move
    excess waits onto single-wait NoOps prepended on the same engine."""
    n = 0
    for fn in nc.m.functions:
        for bb in fn.blocks:
            newlist = []
            changed = False
            for inst in bb.instructions:
                si = inst.sync_info
                if si is not None and len(si.on_wait) > maxw:
                    waits = list(si.on_wait)
                    pre, keep = waits[:-maxw], waits[-maxw:]
                    for i in range(0, len(pre), maxw):
                        n += 1
                        d = mybir.InstNoOp(name=f"SWX{n}", ins=[], outs=[])
                        d.engine = inst.engine
                        d.sync_info = mybir.SyncInfo(on_wait=pre[i : i + maxw], on_update=[])
                        newlist.append(d)
                    inst.sync_info = mybir.SyncInfo(on_wait=keep, on_update=list(si.on_update))
                    changed = True
                newlist.append(inst)
            if changed:
                bb.instructions = newlist
    return n


def build_nc(nb=NB, split=True):
    nc = bass.Bass("TRN2", target_bir_lowering=False, debug=False, num_devices=1)

    x_d = nc.dram_tensor("x", [nb * TOK, F], F32R, kind="ExternalInput")
    w_d = {m: nc.dram_tensor(f"w{m}", [F, F], F32R, kind="ExternalInput")
           for m in ("q", "k", "v", "o")}
    bq_d = nc.dram_tensor("bq", [128, 2], F32, kind="ExternalInput")
    bk_d = nc.dram_tensor("bk", [128, 2], F32, kind="ExternalInput")
    co_d = nc.dram_tensor("co", [128, 2], F32, kind="ExternalInput")
    corep_d = nc.dram_tensor("corep", [128, F], F32, kind="ExternalInput")
    idf_d = nc.dram_tensor("idf", [128, 128], F32, kind="ExternalInput")
    idb_d = nc.dram_tensor("idb", [128, 128], BF16, kind="ExternalInput")
    out_d = nc.dram_tensor("out", [nb * TOK, F], F32, kind="ExternalOutput")

    with tile.TileContext(nc) as tc:
        with (
            tc.tile_pool(name="const", bufs=1) as cpool,
            tc.tile_pool(name="slab", bufs=3) as slab_pool,
            tc.tile_pool(name="big", bufs=1) as big,
            tc.tile_pool(name="att", bufs=4) as att,
            tc.tile_pool(name="outp", bufs=4) as outp,
            tc.tile_pool(name="ps", bufs=8, space="PSUM") as ps,
        ):
            # ---- constants ----
            w = {}
            for m in ("q", "k", "v", "o"):
                for c in range(2):
                    t = cpool.tile([128, F], F32R, tag=f"w{m}{c}", name=f"w{m}{c}")
                    nc.sync.dma_start(t[:], w_d[m][128 * c : 128 * (c + 1), :])
                    w[m, c] = t
            bq = cpool.tile([128, 2], F32, tag="bq", name="bq_sb")
            nc.sync.dma_start(bq[:], bq_d[:])
            bk = cpool.tile([128, 2], F32, tag="bk", name="bk_sb")
            nc.sync.dma_start(bk[:], bk_d[:])
            co = cpool.tile([128, 2], F32, tag="co", name="co_sb")
            nc.sync.dma_start(co[:], co_d[:])
            corep = cpool.tile([128, F], F32, tag="corep", name="corep_sb")
            nc.sync.dma_start(corep[:], corep_d[:])
            idf = cpool.tile([128, 128], F32, tag="idf", name="idf_sb")
            nc.sync.dma_start(idf[:], idf_d[:])
            idb = cpool.tile([128, 128], BF16, tag="idb", name="idb_sb")
            nc.sync.dma_start(idb[:], idb_d[:])

            def pair_ap(t128, p):
                # [128, 4096] -> [128, 2, 64] AP selecting time-major group
                # pair p (cols {64t + n} for n in {2p, 2p+1}), n outer.
                return t128[:, 0:TOK].rearrange("q (t n) -> q n t", n=N)[:, 2 * p : 2 * p + 2, :]

            for b in range(nb):
                # ---- stage A: load + transpose to feature-major ----
                xt = [big.tile([128, TOK], F32R, tag=f"xt{c}", name=f"xt{c}_{b}") for c in range(2)]
                for s in range(4):
                    xs_slab = slab_pool.tile([128, 2048], F32R, tag="slab", name=f"slab_{b}_{s}")
                    src = x_d[b * TOK + 1024 * s : b * TOK + 1024 * (s + 1), :]
                    nc.sync.dma_start(xs_slab[:], src.rearrange("(i p) f -> p i f", p=128))
                    for i in range(8):
                        t128 = 8 * s + i
                        for c in range(2):
                            pt = ps.tile([128, 128], F32, tag="ps", name=f"pst_{b}_{s}_{i}_{c}")
                            nc.tensor.matmul(
                                pt[:],
                                xs_slab[:, 256 * i + 128 * c : 256 * i + 128 * (c + 1)].bitcast(F32),
                                idf[:], is_transpose=True, start=True, stop=True,
                            )
                            dst = xt[c][:, 128 * t128 : 128 * (t128 + 1)]
                            if (t128 + c) % 2 == 0:
                                nc.scalar.copy(dst, pt[:])
                            else:
                                nc.vector.tensor_copy(dst, pt[:])

                xs = [big.tile([128, TOK], F32R, tag=f"xs{c}", name=f"xs{c}_{b}") for c in range(2)]

                for layer in range(2):
                    src_t = xt if layer == 0 else xs
                    # ---- Q/K projections -> bf16 (weight-stationary) ----
                    qt = [big.tile([128, TOK], BF16, tag=f"qt{c}", name=f"qt{c}_{b}_{layer}") for c in range(2)]
                    kt = [big.tile([128, TOK], BF16, tag=f"kt{c}", name=f"kt{c}_{b}_{layer}") for c in range(2)]
                    for g in range(2):
                        for s in range(8):
                            sl = slice(512 * s, 512 * (s + 1))
                            pq = ps.tile([128, 512], F32, tag="ps", name=f"psq_{b}_{layer}_{g}_{s}")
                            for c in range(2):
                                nc.tensor.matmul(
                                    pq[:], w["q", c][:, 128 * g : 128 * (g + 1)],
                                    src_t[c][:, sl], start=(c == 0), stop=(c == 1),
                                )
                            nc.scalar.activation(qt[g][:, sl], pq[:], AF.Identity,
                                                 bias=bq[:, g : g + 1])
                            pk = ps.tile([128, 512], F32, tag="ps", name=f"psk_{b}_{layer}_{g}_{s}")
                            for c in range(2):
                                nc.tensor.matmul(
                                    pk[:], w["k", c][:, 128 * g : 128 * (g + 1)],
                                    src_t[c][:, sl], start=(c == 0), stop=(c == 1),
                                )
                            nc.vector.tensor_scalar_add(kt[g][:, sl], pk[:], bk[:, g : g + 1])

                    # ---- V token-major (activation-stationary) ----
                    if layer == 0:
                        vt = big.tile([128, 2 * TOK], BF16, tag="vt", name=f"vt_{b}_{layer}")
                        for p in range(NPAIR):
                            pv = ps.tile([128, 256], F32, tag="ps", name=f"psv_{b}_{layer}_{p}")
                            for c in range(2):
                                nc.tensor.matmul(pv[:], src_t[c][:, 128 * p : 128 * (p + 1)],
                                                 w["v", c][:], start=(c == 0), stop=(c == 1))
                            dst = vt[:, 256 * p : 256 * (p + 1)]
                            if p % 2 == 0:
                                nc.scalar.copy(dst, pv[:])
                            else:
                                nc.vector.tensor_copy(dst, pv[:])
                    else:
                        vt = big.tile([64, 4 * TOK], BF16, tag="vt", name=f"vt_{b}_{layer}")
                        for gi in range(N):
                            pv = ps.tile([64, 256], F32, tag="ps", name=f"psv_{b}_{layer}_{gi}")
                            for c in range(2):
                                nc.tensor.matmul(pv[:], src_t[c][:, gi : TOK : N],
                                                 w["v", c][:], start=(c == 0), stop=(c == 1))
                            dst = vt[:, 256 * gi : 256 * (gi + 1)]
                            if gi % 2 == 0:
                                nc.scalar.copy(dst, pv[:])
                            else:
                                nc.vector.tensor_copy(dst, pv[:])

                    # ---- attention pairs ----
                    at = [big.tile([128, TOK], F32R, tag=f"at{c}", name=f"at{c}_{b}_{layer}") for c in range(2)]
                    if layer == 0:
                      for p in range(NPAIR):
                        sp = ps.tile([128, 128], F32, tag="ps", name=f"pss_{b}_{layer}_{p}")
                        for c in range(2):
                            nc.tensor.matmul(sp[:], qt[c][:, 128 * p : 128 * (p + 1)],
                                             kt[c][:, 128 * p : 128 * (p + 1)],
                                             start=(c == 0), stop=(c == 1))
                        psb = att.tile([128, 128], BF16, tag="p", name=f"psb_{b}_{layer}_{p}")
                        sums = att.tile([128, 1], F32, tag="sums", name=f"sums_{b}_{layer}_{p}")
                        rcp = att.tile([128, 1], F32, tag="rcp", name=f"rcp_{b}_{layer}_{p}")
                        for h in range(2):
                            blk = slice(64 * h, 64 * (h + 1))
                            nc.scalar.activation(psb[blk, blk], sp[blk, blk], AF.Exp,
                                                 accum_out=sums[blk, 0:1])
                        nc.gpsimd.memset(psb[0:64, 64:128], 0.0)
                        nc.gpsimd.memset(psb[64:128, 0:64], 0.0)
                        nc.vector.reciprocal(rcp[:], sums[:])
                        for h in range(2):
                            blk = slice(64 * h, 64 * (h + 1))
                            nc.vector.tensor_scalar_mul(psb[blk, blk], psb[blk, blk],
                                                        rcp[blk, 0:1])
                        ptp = ps.tile([128, 128], BF16, tag="ps", name=f"psp_{b}_{layer}_{p}")
                        nc.tensor.matmul(ptp[:], psb[:], idb[:], is_transpose=True,
                                         start=True, stop=True)
                        pts = att.tile([128, 128], BF16, tag="pt", name=f"pts_{b}_{layer}_{p}")
                        nc.vector.tensor_copy(pts[:], ptp[:])
                        for c in range(2):
                            pa = ps.tile([128, 128], F32, tag="ps", name=f"psa_{b}_{layer}_{p}_{c}")
                            nc.tensor.matmul(
                                pa[:], vt[:, 256 * p + 128 * c : 256 * p + 128 * (c + 1)],
                                pts[:], start=True, stop=True,
                            )
                            dst = at[c][:, 128 * p : 128 * (p + 1)]
                            if (p + c) % 2 == 0:
                                nc.scalar.copy(dst, pa[:])
                            else:
                                nc.vector.tensor_copy(dst, pa[:])
                    else:
                      for gi in range(N):
                        sp = ps.tile([64, 64], F32, tag="ps", name=f"pss_{b}_{layer}_{gi}")
                        for c in range(2):
                            nc.tensor.matmul(sp[:], qt[c][:, gi : TOK : N],
                                             kt[c][:, gi : TOK : N],
                                             start=(c == 0), stop=(c == 1))
                        psb = att.tile([64, 64], BF16, tag="p", name=f"psb_{b}_{layer}_{gi}")
                        sums = att.tile([64, 1], F32, tag="sums", name=f"sums_{b}_{layer}_{gi}")
                        rcp = att.tile([64, 1], F32, tag="rcp", name=f"rcp_{b}_{layer}_{gi}")
                        nc.scalar.activation(psb[:], sp[:], AF.Exp, accum_out=sums[:])
                        nc.vector.reciprocal(rcp[:], sums[:])
                        nc.vector.tensor_scalar_mul(psb[:], psb[:], rcp[:, 0:1])
                        ptp = ps.tile([64, 64], BF16, tag="ps", name=f"psp_{b}_{layer}_{gi}")
                        nc.tensor.matmul(ptp[:], psb[:], idb[0:64, 0:64], is_transpose=True,
                                         start=True, stop=True)
                        pts = att.tile([64, 64], BF16, tag="pt", name=f"pts_{b}_{layer}_{gi}")
                        nc.vector.tensor_copy(pts[:], ptp[:])
                        for c in range(2):
                            pa = ps.tile([128, 64], F32, tag="ps", name=f"psa_{b}_{layer}_{gi}_{c}")
                            nc.tensor.matmul(
                                pa[:], vt[:, 256 * gi + 128 * c : 256 * gi + 128 * (c + 1)],
                                pts[:], start=True, stop=True,
                            )
                            dst = at[c][:, 64 * gi : 64 * (gi + 1)]
                            if (gi + c) % 2 == 0:
                                nc.scalar.copy(dst, pa[:])
                            else:
                                nc.vector.tensor_copy(dst, pa[:])
                    if layer == 0:
                        # ---- O-projection + residual -> xsT (feature-major) ----
                        for g in range(2):
                            for s in range(8):
                                sl = slice(512 * s, 512 * (s + 1))
                                po = ps.tile([128, 512], F32, tag="ps", name=f"pso_{b}_{g}_{s}")
                                for c in range(2):
                                    nc.tensor.matmul(
                                        po[:], w["o", c][:, 128 * g : 128 * (g + 1)],
                                        at[c][:, sl], start=(c == 0), stop=(c == 1),
                                    )
                                nc.scalar.activation(xs[g][:, sl], po[:], AF.Identity,
                                                     bias=co[:, g : g + 1])
                                nc.vector.tensor_add(xs[g][:, sl], xs[g][:, sl],
                                                     xt[g][:, sl].bitcast(F32))
                    else:
                        # ---- final: A-stationary O-proj + transposed residual ----
                        for p in range(NPAIR):
                          for h in range(2):
                            n_idx = 2 * p + h
                            po = ps.tile([64, 256], F32, tag="ps", name=f"pso2_{b}_{p}_{h}")
                            # bracket: full-width O-proj opens/closes the PSUM
                            # group around the two residual transposes
                            nc.tensor.matmul(po[:], at[0][:, 128 * p + 64 * h : 128 * p + 64 * (h + 1)],
                                             w["o", 0][:], start=True, stop=False)
                            for c in range(2):
                                nc.tensor.matmul(
                                    po[:, 128 * c : 128 * (c + 1)],
                                    xs[c][:, n_idx : TOK : N].bitcast(F32), idf[:],
                                    is_transpose=True, start=False, stop=False,
                                )
                            nc.tensor.matmul(po[:], at[1][:, 128 * p + 64 * h : 128 * p + 64 * (h + 1)],
                                             w["o", 1][:], start=False, stop=True)
                            osb = outp.tile([64, 256], F32, tag="osb", name=f"osb_{b}_{p}_{h}")
                            nc.vector.tensor_add(osb[:], po[:], corep[0:64, :])
                            dst = out_d[b * TOK : (b + 1) * TOK, :].rearrange(
                                "(t n) f -> n t f", n=N)[n_idx : n_idx + 1, :, :]
                            nc.sync.dma_start(dst, osb[:])

    if split:
        _split_waits(nc)
    return nc


_NC_CACHE = {}


def _get_nc(nb=NB):
    if nb not in _NC_CACHE:
        _NC_CACHE[nb] = build_nc(nb)
    return _NC_CACHE[nb]


def _host_consts(Wq, bq, Wk, bk, Wv, bv, Wo, bo):
    scale = 0.125  # 1/sqrt(64)
    wq_t = np.ascontiguousarray(Wq.T) * scale
    wk_t = np.ascontiguousarray(Wk.T)
    wv_t = np.ascontiguousarray(Wv.T)
    wo_t = np.ascontiguousarray(Wo.T)
    co_vec = bo + Wo @ bv          # bv commutes through softmax-weighted sum
    bq_s = (bq * scale).reshape(2, 128).T.copy()
    bk_s = bk.reshape(2, 128).T.copy()
    co_s = co_vec.reshape(2, 128).T.copy()
    return {
        "wq": wq_t.astype(np.float32), "wk": wk_t.astype(np.float32),
        "wv": wv_t.astype(np.float32), "wo": wo_t.astype(np.float32),
        "bq": bq_s.astype(np.float32), "bk": bk_s.astype(np.float32),
        "co": co_s.astype(np.float32),
        "corep": np.repeat(co_vec.reshape(1, F), 128, 0).astype(np.float32),
        "idf": np.eye(128, dtype=np.float32),
        "idb": np.eye(128, dtype=ml_dtypes.bfloat16),
    }


def kernel(x, Wq, bq, Wk, bk, Wv, bv, Wo, bo):
    from concourse.bass_utils import run_bass_kernel_spmd

    x = np.asarray(x, dtype=np.float32)
    consts = _host_consts(np.asarray(Wq), np.asarray(bq), np.asarray(Wk),
                          np.asarray(bk), np.asarray(Wv), np.asarray(bv),
                          np.asarray(Wo), np.asarray(bo))
    nc = _get_nc(NB)
    xr = x.reshape(B_FULL, TOK, F)
    in_maps = []
    for i in range(N_CORES):
        m = dict(consts)
        m["x"] = np.ascontiguousarray(xr[NB * i : NB * (i + 1)].reshape(NB * TOK, F))
        in_maps.append(m)
    res = run_bass_kernel_spmd(nc, in_maps, core_ids=list(range(N_CORES)))
    out = np.concatenate([res.results[i]["out"] for i in range(N_CORES)], axis=0)
    return out.reshape(B_FULL, T, N, F)
